# revision 1
# baseline (speedup 1.0000x reference)
"""Conv2d 3x3 (stride 1, pad 1) Trainium2 Bass kernel.

Problem: x (32, 128, 56, 56) fp32, kernels (256, 128, 3, 3) fp32, b (256,) fp32
-> out (32, 256, 56, 56) fp32.

Strategy:
  - Data-parallel over batch: 32 images / 8 cores = 4 images per core. SPMD,
    no collectives.
  - Per core: contraction dim C_in=128 lives on SBUF partitions. The 3x3 conv
    is 9 shifted [128c_in x 128c_out x 448] matmuls accumulated in PSUM (one
    per kernel tap), x held zero-padded in SBUF as [128, 58, 58] so every tap
    is a strided view.
  - Output tiled as [c_out half (128 partitions), 8 rows x 56 cols = 448 free]
    (<= 512 fp32, one PSUM bank). 2 halves x 7 row blocks x 4 images = 56
    accumulation groups of 9 matmuls each per core.
  - Inputs cast to bf16 on host (PE runs 2x the fp32 rate; accumulation stays
    fp32 in PSUM). Bias added during PSUM->SBUF eviction on ScalarE.
"""

import numpy as np
import ml_dtypes

import concourse.bass as bass
import concourse.tile as tile
from concourse import bacc, mybir
from concourse.bass_utils import run_bass_kernel_spmd

N_CORES = 8
N_FULL = 32
N_PER = N_FULL // N_CORES  # 4 images per core
C_IN = 128
C_OUT = 256
H = W = 56
HW = H * W
KS = 3
HP = H + 2  # 58, zero-padded
WP = W + 2
R = 8              # output rows per matmul group
NB = H // R        # 7 row blocks
NFREE = R * W      # 448 <= 512 (one PSUM bank of fp32)

_DT = mybir.dt.bfloat16


def _build():
    nc = bacc.Bacc(
        "TRN2",
        target_bir_lowering=False,
        debug=False,
        num_devices=N_CORES,
    )
    xs = nc.dram_tensor("xs", [N_PER, C_IN, H, W], _DT, kind="ExternalInput").ap()
    wt = nc.dram_tensor("wt", [C_IN, KS * KS * C_OUT], _DT, kind="ExternalInput").ap()
    bt = nc.dram_tensor("bt", [128, 2], mybir.dt.float32, kind="ExternalInput").ap()
    y = nc.dram_tensor(
        "y", [N_PER, C_OUT, HW], mybir.dt.float32, kind="ExternalOutput"
    ).ap()

    with tile.TileContext(nc) as tc:
        with (
            tc.tile_pool(name="const", bufs=1) as const,
            tc.tile_pool(name="xpool", bufs=N_PER) as xpool,
            tc.tile_pool(name="pspool", bufs=8, space="PSUM") as pspool,
            tc.tile_pool(name="opool", bufs=4) as opool,
        ):
            wt_sb = const.tile([C_IN, KS * KS * C_OUT], _DT)
            nc.sync.dma_start(out=wt_sb[:], in_=wt)
            bias_sb = const.tile([128, 2], mybir.dt.float32)
            nc.sync.dma_start(out=bias_sb[:], in_=bt)

            for n in range(N_PER):
                xp = xpool.tile([C_IN, HP, WP], _DT, tag="xp")
                nc.gpsimd.memset(xp[:], 0.0)
                nc.sync.dma_start(
                    out=xp[:, 1 : H + 1, 1 : W + 1], in_=xs[n]
                )
                for half in range(2):
                    for rb in range(NB):
                        ps = pspool.tile([128, NFREE], mybir.dt.float32, tag="ps")
                        r0 = rb * R
                        for kh in range(KS):
                            for kw in range(KS):
                                off = (kh * KS + kw) * C_OUT + half * 128
                                nc.tensor.matmul(
                                    ps[:],
                                    lhsT=wt_sb[:, off : off + 128],
                                    rhs=xp[:, r0 + kh : r0 + kh + R, kw : kw + W],
                                    start=(kh == 0 and kw == 0),
                                    stop=(kh == KS - 1 and kw == KS - 1),
                                )
                        ot = opool.tile([128, NFREE], mybir.dt.float32, tag="ot")
                        nc.scalar.activation(
                            ot[:],
                            ps[:],
                            mybir.ActivationFunctionType.Identity,
                            bias=bias_sb[:, half : half + 1],
                            scale=1.0,
                        )
                        nc.sync.dma_start(
                            out=y[
                                n,
                                half * 128 : (half + 1) * 128,
                                rb * NFREE : (rb + 1) * NFREE,
                            ],
                            in_=ot[:],
                        )
    nc.compile()
    return nc


_NC = None


def _get_nc():
    global _NC
    if _NC is None:
        _NC = _build()
    return _NC


def _prep_inputs(x, kernels, b):
    bf16 = ml_dtypes.bfloat16
    xb = np.ascontiguousarray(x, dtype=np.float32).astype(bf16)
    # [O, I, kh, kw] -> [I, kh, kw, O] -> [128, 9*256]
    wtb = (
        np.ascontiguousarray(np.transpose(kernels, (1, 2, 3, 0)))
        .reshape(C_IN, KS * KS * C_OUT)
        .astype(bf16)
    )
    # bias [256] -> [128, 2]: column h holds b[h*128 : (h+1)*128]
    btb = np.ascontiguousarray(
        np.asarray(b, dtype=np.float32).reshape(2, 128).T
    )
    return xb, wtb, btb


def kernel(x, kernels, b):
    nc = _get_nc()
    xb, wtb, btb = _prep_inputs(x, kernels, b)
    in_maps = [
        {"xs": xb[i * N_PER : (i + 1) * N_PER], "wt": wtb, "bt": btb}
        for i in range(N_CORES)
    ]
    res = run_bass_kernel_spmd(nc, in_maps, core_ids=list(range(N_CORES)))
    out = np.concatenate(
        [r["y"].reshape(N_PER, C_OUT, H, W) for r in res.results], axis=0
    )
    return np.ascontiguousarray(out, dtype=np.float32)


# revision 2
# speedup vs baseline: 1.1342x; 1.1342x over previous
"""Conv2d 3x3 (stride 1, pad 1) Trainium2 Bass kernel.

Problem: x (32, 128, 56, 56) fp32, kernels (256, 128, 3, 3) fp32, b (256,) fp32
-> out (32, 256, 56, 56) fp32.

Strategy:
  - Data-parallel over batch: 32 images / 8 cores = 4 images per core. SPMD,
    no collectives.
  - Per core: contraction dim C_in=128 lives on SBUF partitions. The 3x3 conv
    is 9 shifted [128c_in x 128c_out x 448] matmuls accumulated in PSUM (one
    per kernel tap), x held zero-padded in SBUF as [128, 58, 58] so every tap
    is a strided view.
  - Output tiled as [c_out half (128 partitions), 8 rows x 56 cols = 448 free]
    (<= 512 fp32, one PSUM bank). 2 halves x 7 row blocks x 4 images = 56
    accumulation groups of 9 matmuls each per core.
  - Inputs cast to bf16 on host (PE runs 2x the fp32 rate; accumulation stays
    fp32 in PSUM). Bias added during PSUM->SBUF eviction on ScalarE.
"""

import numpy as np
import ml_dtypes

import concourse.bass as bass
import concourse.tile as tile
from concourse import bacc, mybir
from concourse.bass_utils import run_bass_kernel_spmd

N_CORES = 8
N_FULL = 32
N_PER = N_FULL // N_CORES  # 4 images per core
C_IN = 128
C_OUT = 256
H = W = 56
HW = H * W
KS = 3
HP = H + 2  # 58, zero-padded
WP = W + 2
R = 8              # output rows per matmul group
NB = H // R        # 7 row blocks
NFREE = R * W      # 448 <= 512 (one PSUM bank of fp32)

_DT = mybir.dt.bfloat16


def _build():
    nc = bacc.Bacc(
        "TRN2",
        target_bir_lowering=False,
        debug=False,
        num_devices=N_CORES,
    )
    xs = nc.dram_tensor("xs", [N_PER, C_IN, H, W], _DT, kind="ExternalInput").ap()
    wt = nc.dram_tensor("wt", [C_IN, KS * KS * C_OUT], _DT, kind="ExternalInput").ap()
    bt = nc.dram_tensor("bt", [128, 2], mybir.dt.float32, kind="ExternalInput").ap()
    y = nc.dram_tensor(
        "y", [N_PER, C_OUT, HW], mybir.dt.float32, kind="ExternalOutput"
    ).ap()

    with tile.TileContext(nc) as tc:
        with (
            tc.tile_pool(name="const", bufs=1) as const,
            tc.tile_pool(name="xpool", bufs=N_PER) as xpool,
            tc.tile_pool(name="pspool", bufs=8, space="PSUM") as pspool,
            tc.tile_pool(name="opool", bufs=4) as opool,
        ):
            wt_sb = const.tile([C_IN, KS * KS * C_OUT], _DT)
            nc.sync.dma_start(out=wt_sb[:], in_=wt)
            bias_sb = const.tile([128, 2], mybir.dt.float32)
            nc.sync.dma_start(out=bias_sb[:], in_=bt)

            # Tap order: center tap (kh=1, kw=1) first — it writes the FULL
            # [128, 448] window, so start=True initializes every PSUM element
            # and the ragged boundary taps can accumulate into sub-windows.
            taps = [(1, 1)] + [
                (kh, kw)
                for kh in range(KS)
                for kw in range(KS)
                if not (kh == 1 and kw == 1)
            ]
            for n in range(N_PER):
                xu = xpool.tile([C_IN, H, W], _DT, tag="xp")
                nc.sync.dma_start(out=xu[:], in_=xs[n])
                for half in range(2):
                    for rb in range(NB):
                        ps = pspool.tile([128, NFREE], mybir.dt.float32, tag="ps")
                        ps3 = ps[:].rearrange("p (r c) -> p r c", r=R)
                        r0 = rb * R
                        for idx, (kh, kw) in enumerate(taps):
                            dh, dw = kh - 1, kw - 1
                            rlo = max(r0, -dh)
                            rhi = min(r0 + R, H - dh)
                            clo = max(0, -dw)
                            chi = min(W, W - dw)
                            off = (kh * KS + kw) * C_OUT + half * 128
                            nc.tensor.matmul(
                                ps3[:, rlo - r0 : rhi - r0, clo:chi],
                                lhsT=wt_sb[:, off : off + 128],
                                rhs=xu[:, rlo + dh : rhi + dh, clo + dw : chi + dw],
                                start=(idx == 0),
                                stop=(idx == len(taps) - 1),
                            )
                        ot = opool.tile([128, NFREE], mybir.dt.float32, tag="ot")
                        nc.scalar.activation(
                            ot[:],
                            ps[:],
                            mybir.ActivationFunctionType.Identity,
                            bias=bias_sb[:, half : half + 1],
                            scale=1.0,
                        )
                        nc.sync.dma_start(
                            out=y[
                                n,
                                half * 128 : (half + 1) * 128,
                                rb * NFREE : (rb + 1) * NFREE,
                            ],
                            in_=ot[:],
                        )
    nc.compile()
    return nc


_NC = None


def _get_nc():
    global _NC
    if _NC is None:
        _NC = _build()
    return _NC


def _prep_inputs(x, kernels, b):
    bf16 = ml_dtypes.bfloat16
    xb = np.ascontiguousarray(x, dtype=np.float32).astype(bf16)
    # [O, I, kh, kw] -> [I, kh, kw, O] -> [128, 9*256]
    wtb = (
        np.ascontiguousarray(np.transpose(kernels, (1, 2, 3, 0)))
        .reshape(C_IN, KS * KS * C_OUT)
        .astype(bf16)
    )
    # bias [256] -> [128, 2]: column h holds b[h*128 : (h+1)*128]
    btb = np.ascontiguousarray(
        np.asarray(b, dtype=np.float32).reshape(2, 128).T
    )
    return xb, wtb, btb


def kernel(x, kernels, b):
    nc = _get_nc()
    xb, wtb, btb = _prep_inputs(x, kernels, b)
    in_maps = [
        {"xs": xb[i * N_PER : (i + 1) * N_PER], "wt": wtb, "bt": btb}
        for i in range(N_CORES)
    ]
    res = run_bass_kernel_spmd(nc, in_maps, core_ids=list(range(N_CORES)))
    out = np.concatenate(
        [r["y"].reshape(N_PER, C_OUT, H, W) for r in res.results], axis=0
    )
    return np.ascontiguousarray(out, dtype=np.float32)


# revision 5
# speedup vs baseline: 1.1564x; 1.0196x over previous
"""Conv2d 3x3 (stride 1, pad 1) Trainium2 Bass kernel.

Problem: x (32, 128, 56, 56) fp32, kernels (256, 128, 3, 3) fp32, b (256,) fp32
-> out (32, 256, 56, 56) fp32.

Strategy:
  - Data-parallel over batch: 32 images / 8 cores = 4 images per core. SPMD,
    no collectives.
  - Per core: contraction dim C_in=128 lives on SBUF partitions. The 3x3 conv
    is 9 shifted [128c_in x 128c_out x 448] matmuls accumulated in PSUM (one
    per kernel tap), x held zero-padded in SBUF as [128, 58, 58] so every tap
    is a strided view.
  - Output tiled as [c_out half (128 partitions), 8 rows x 56 cols = 448 free]
    (<= 512 fp32, one PSUM bank). 2 halves x 7 row blocks x 4 images = 56
    accumulation groups of 9 matmuls each per core.
  - Inputs cast to bf16 on host (PE runs 2x the fp32 rate; accumulation stays
    fp32 in PSUM). Bias added during PSUM->SBUF eviction on ScalarE.
"""

import numpy as np
import ml_dtypes

import concourse.bass as bass
import concourse.tile as tile
from concourse import bacc, mybir
from concourse.bass_utils import run_bass_kernel_spmd

N_CORES = 8
N_FULL = 32
N_PER = N_FULL // N_CORES  # 4 images per core
C_IN = 128
C_OUT = 256
H = W = 56
HW = H * W
KS = 3
HP = H + 2  # 58, zero-padded
WP = W + 2
R = 8              # output rows per matmul group
NB = H // R        # 7 row blocks
NFREE = R * W      # 448 <= 512 (one PSUM bank of fp32)

_DT = mybir.dt.bfloat16


def _build():
    nc = bacc.Bacc(
        "TRN2",
        target_bir_lowering=False,
        debug=False,
        num_devices=N_CORES,
    )
    xs = nc.dram_tensor("xs", [N_PER, C_IN, H, W], _DT, kind="ExternalInput").ap()
    wt = nc.dram_tensor("wt", [C_IN, KS * KS * C_OUT], _DT, kind="ExternalInput").ap()
    bt = nc.dram_tensor("bt", [128, 2], mybir.dt.float32, kind="ExternalInput").ap()
    y = nc.dram_tensor(
        "y", [N_PER, C_OUT, HW], mybir.dt.float32, kind="ExternalOutput"
    ).ap()

    with tile.TileContext(nc) as tc:
        with (
            tc.tile_pool(name="const", bufs=1) as const,
            tc.tile_pool(name="xpool", bufs=N_PER * NB) as xpool,
            tc.tile_pool(name="pspool", bufs=8, space="PSUM") as pspool,
            tc.tile_pool(name="opool", bufs=4) as opool,
        ):
            # weights/bias on the gpsimd queue so they don't serialize ahead
            # of the first x chunk on sync's queue
            wt_sb = const.tile([C_IN, KS * KS * C_OUT], _DT)
            nc.gpsimd.dma_start(out=wt_sb[:], in_=wt)
            bias_sb = const.tile([128, 2], mybir.dt.float32)
            nc.gpsimd.dma_start(out=bias_sb[:], in_=bt)

            # Tap order: center tap (kh=1, kw=1) first — it writes the FULL
            # [128, 448] window, so start=True initializes every PSUM element
            # and the ragged boundary taps can accumulate into sub-windows.
            taps = [(1, 1)] + [
                (kh, kw)
                for kh in range(KS)
                for kw in range(KS)
                if not (kh == 1 and kw == 1)
            ]
            for n in range(N_PER):
                # 7 row-chunks per image with 1-row halos: chunk c holds input
                # rows (8c-1 .. 8c+9), i.e. tile row t <-> input row 8c-1+t.
                # The first matmul group only waits for its own ~143KB chunk.
                chunks = []
                for c in range(NB):
                    xc = xpool.tile([C_IN, R + 2, W], _DT, tag="xc")
                    lo = max(0, c * R - 1)
                    hi = min(H, c * R + R + 1)
                    nc.sync.dma_start(
                        out=xc[:, lo - (c * R - 1) : hi - (c * R - 1), :],
                        in_=xs[n, :, lo:hi, :],
                    )
                    chunks.append(xc)
                for half in range(2):
                    for rb in range(NB):
                        xc = chunks[rb]
                        ps = pspool.tile([128, NFREE], mybir.dt.float32, tag="ps")
                        ps3 = ps[:].rearrange("p (r c) -> p r c", r=R)
                        r0 = rb * R
                        for idx, (kh, kw) in enumerate(taps):
                            dh, dw = kh - 1, kw - 1
                            rlo = max(r0, -dh)
                            rhi = min(r0 + R, H - dh)
                            clo = max(0, -dw)
                            chi = min(W, W - dw)
                            off = (kh * KS + kw) * C_OUT + half * 128
                            nc.tensor.matmul(
                                ps3[:, rlo - r0 : rhi - r0, clo:chi],
                                lhsT=wt_sb[:, off : off + 128],
                                rhs=xc[
                                    :,
                                    rlo + dh - r0 + 1 : rhi + dh - r0 + 1,
                                    clo + dw : chi + dw,
                                ],
                                start=(idx == 0),
                                stop=(idx == len(taps) - 1),
                            )
                        ot = opool.tile([128, NFREE], mybir.dt.float32, tag="ot")
                        nc.scalar.activation(
                            ot[:],
                            ps[:],
                            mybir.ActivationFunctionType.Identity,
                            bias=bias_sb[:, half : half + 1],
                            scale=1.0,
                        )
                        nc.scalar.dma_start(
                            out=y[
                                n,
                                half * 128 : (half + 1) * 128,
                                rb * NFREE : (rb + 1) * NFREE,
                            ],
                            in_=ot[:],
                        )
    nc.compile()
    return nc


_NC = None


def _get_nc():
    global _NC
    if _NC is None:
        _NC = _build()
    return _NC


def _prep_inputs(x, kernels, b):
    bf16 = ml_dtypes.bfloat16
    xb = np.ascontiguousarray(x, dtype=np.float32).astype(bf16)
    # [O, I, kh, kw] -> [I, kh, kw, O] -> [128, 9*256]
    wtb = (
        np.ascontiguousarray(np.transpose(kernels, (1, 2, 3, 0)))
        .reshape(C_IN, KS * KS * C_OUT)
        .astype(bf16)
    )
    # bias [256] -> [128, 2]: column h holds b[h*128 : (h+1)*128]
    btb = np.ascontiguousarray(
        np.asarray(b, dtype=np.float32).reshape(2, 128).T
    )
    return xb, wtb, btb


def kernel(x, kernels, b):
    nc = _get_nc()
    xb, wtb, btb = _prep_inputs(x, kernels, b)
    in_maps = [
        {"xs": xb[i * N_PER : (i + 1) * N_PER], "wt": wtb, "bt": btb}
        for i in range(N_CORES)
    ]
    res = run_bass_kernel_spmd(nc, in_maps, core_ids=list(range(N_CORES)))
    out = np.concatenate(
        [r["y"].reshape(N_PER, C_OUT, H, W) for r in res.results], axis=0
    )
    return np.ascontiguousarray(out, dtype=np.float32)


# revision 10
# speedup vs baseline: 1.1755x; 1.0165x over previous
"""Conv2d 3x3 (stride 1, pad 1) Trainium2 Bass kernel.

Problem: x (32, 128, 56, 56) fp32, kernels (256, 128, 3, 3) fp32, b (256,) fp32
-> out (32, 256, 56, 56) fp32.

Strategy:
  - Data-parallel over batch: 32 images / 8 cores = 4 images per core. SPMD,
    no collectives.
  - Per core: contraction dim C_in=128 lives on SBUF partitions. The 3x3 conv
    is 9 shifted [128c_in x 128c_out x 448] matmuls accumulated in PSUM (one
    per kernel tap), x held zero-padded in SBUF as [128, 58, 58] so every tap
    is a strided view.
  - Output tiled as [c_out half (128 partitions), 8 rows x 56 cols = 448 free]
    (<= 512 fp32, one PSUM bank). 2 halves x 7 row blocks x 4 images = 56
    accumulation groups of 9 matmuls each per core.
  - Inputs cast to bf16 on host (PE runs 2x the fp32 rate; accumulation stays
    fp32 in PSUM). Bias added during PSUM->SBUF eviction on ScalarE.
"""

import numpy as np
import ml_dtypes

import concourse.bass as bass
import concourse.tile as tile
from concourse import bacc, mybir
from concourse.bass_utils import run_bass_kernel_spmd

N_CORES = 8
N_FULL = 32
N_PER = N_FULL // N_CORES  # 4 images per core
C_IN = 128
C_OUT = 256
H = W = 56
HW = H * W
KS = 3
HP = H + 2  # 58, zero-padded
WP = W + 2
R = 8              # output rows per matmul group
NB = H // R        # 7 row blocks
NFREE = R * W      # 448 <= 512 (one PSUM bank of fp32)

_DT = mybir.dt.bfloat16

# Tap order: center tap (kh=1, kw=1) first — it writes the FULL [128, 448]
# window, so start=True initializes every PSUM element and the ragged
# boundary taps can accumulate into sub-windows. Host-side weight layout
# follows this order so the first weight-DMA part covers the first taps.
TAPS = [(1, 1)] + [
    (kh, kw) for kh in range(KS) for kw in range(KS) if not (kh == 1 and kw == 1)
]
N_WPART = 3  # weights split into 3 DMAs of 3 taps each


def _build():
    nc = bacc.Bacc(
        "TRN2",
        target_bir_lowering=False,
        debug=False,
        num_devices=N_CORES,
    )
    xs = nc.dram_tensor("xs", [N_PER, C_IN, H, W], _DT, kind="ExternalInput").ap()
    wt = nc.dram_tensor("wt", [C_IN, KS * KS * C_OUT], _DT, kind="ExternalInput").ap()
    bt = nc.dram_tensor("bt", [128, 2], mybir.dt.float32, kind="ExternalInput").ap()
    y = nc.dram_tensor(
        "y", [N_PER, C_OUT, HW], mybir.dt.float32, kind="ExternalOutput"
    ).ap()

    tap_cols = KS * KS * C_OUT // N_WPART  # 768 columns per weight part

    with tile.TileContext(nc) as tc:
        with (
            tc.tile_pool(name="const", bufs=1) as const,
            tc.tile_pool(name="wpool", bufs=1, space="PSUM") as wpool,
            tc.tile_pool(name="xpool", bufs=N_PER * NB) as xpool,
            tc.tile_pool(name="pspool", bufs=7, space="PSUM") as pspool,
            tc.tile_pool(name="opool", bufs=4) as opool,
        ):
            # PE warm-up: dummy matmuls on a zeroed scratch tile depend on no
            # DMA, so they run during the input-load window and lift the HAM
            # clock gate (1.2 -> 2.4 GHz) before real matmuls arrive.
            warm = const.tile([128, 512], _DT)
            nc.vector.memset(warm[:], 0.0)
            wps = wpool.tile([128, 512], mybir.dt.float32)
            for i in range(6):
                nc.tensor.matmul(
                    wps[:],
                    lhsT=warm[:, :128],
                    rhs=warm[:],
                    start=(i == 0),
                    stop=(i == 5),
                )

            # weights/bias on the gpsimd queue so they don't serialize ahead
            # of the first x chunk on sync's queue; 3 separate tiles so the
            # first matmuls wait only on part 0
            wparts = []
            for p in range(N_WPART):
                wp_sb = const.tile([C_IN, tap_cols], _DT, name=f"wt_sb{p}")
                nc.gpsimd.dma_start(
                    out=wp_sb[:], in_=wt[:, p * tap_cols : (p + 1) * tap_cols]
                )
                wparts.append(wp_sb)
            bias_sb = const.tile([128, 2], mybir.dt.float32)
            nc.gpsimd.dma_start(out=bias_sb[:], in_=bt)

            taps = TAPS
            for n in range(N_PER):
                # 7 row-chunks per image with 1-row halos: chunk c holds input
                # rows (8c-1 .. 8c+9), i.e. tile row t <-> input row 8c-1+t.
                # The first matmul group only waits for its own ~143KB chunk.
                chunks = []
                for c in range(NB):
                    xc = xpool.tile([C_IN, R + 2, W], _DT, tag="xc")
                    lo = max(0, c * R - 1)
                    hi = min(H, c * R + R + 1)
                    nc.sync.dma_start(
                        out=xc[:, lo - (c * R - 1) : hi - (c * R - 1), :],
                        in_=xs[n, :, lo:hi, :],
                    )
                    chunks.append(xc)
                for half in range(2):
                    for rb in range(NB):
                        xc = chunks[rb]
                        ps = pspool.tile([128, NFREE], mybir.dt.float32, tag="ps")
                        ps3 = ps[:].rearrange("p (r c) -> p r c", r=R)
                        r0 = rb * R
                        for idx, (kh, kw) in enumerate(taps):
                            dh, dw = kh - 1, kw - 1
                            rlo = max(r0, -dh)
                            rhi = min(r0 + R, H - dh)
                            clo = max(0, -dw)
                            chi = min(W, W - dw)
                            off = (idx % 3) * C_OUT + half * 128
                            nc.tensor.matmul(
                                ps3[:, rlo - r0 : rhi - r0, clo:chi],
                                lhsT=wparts[idx // 3][:, off : off + 128],
                                rhs=xc[
                                    :,
                                    rlo + dh - r0 + 1 : rhi + dh - r0 + 1,
                                    clo + dw : chi + dw,
                                ],
                                start=(idx == 0),
                                stop=(idx == len(taps) - 1),
                            )
                        ot = opool.tile([128, NFREE], mybir.dt.float32, tag="ot")
                        nc.scalar.activation(
                            ot[:],
                            ps[:],
                            mybir.ActivationFunctionType.Identity,
                            bias=bias_sb[:, half : half + 1],
                            scale=1.0,
                        )
                        nc.sync.dma_start(
                            out=y[
                                n,
                                half * 128 : (half + 1) * 128,
                                rb * NFREE : (rb + 1) * NFREE,
                            ],
                            in_=ot[:],
                        )
    nc.compile()
    return nc


_NC = None


def _get_nc():
    global _NC
    if _NC is None:
        _NC = _build()
    return _NC


def _prep_inputs(x, kernels, b):
    bf16 = ml_dtypes.bfloat16
    xb = np.ascontiguousarray(x, dtype=np.float32).astype(bf16)
    # [O, I, kh, kw] -> [I, tap, O] in TAPS order -> [128, 9*256]
    wk = np.transpose(np.asarray(kernels, dtype=np.float32), (1, 2, 3, 0))
    wtb = np.ascontiguousarray(
        np.stack([wk[:, kh, kw, :] for kh, kw in TAPS], axis=1)
    ).reshape(C_IN, KS * KS * C_OUT).astype(bf16)
    # bias [256] -> [128, 2]: column h holds b[h*128 : (h+1)*128]
    btb = np.ascontiguousarray(
        np.asarray(b, dtype=np.float32).reshape(2, 128).T
    )
    return xb, wtb, btb


def kernel(x, kernels, b):
    nc = _get_nc()
    xb, wtb, btb = _prep_inputs(x, kernels, b)
    in_maps = [
        {"xs": xb[i * N_PER : (i + 1) * N_PER], "wt": wtb, "bt": btb}
        for i in range(N_CORES)
    ]
    res = run_bass_kernel_spmd(nc, in_maps, core_ids=list(range(N_CORES)))
    out = np.concatenate(
        [r["y"].reshape(N_PER, C_OUT, H, W) for r in res.results], axis=0
    )
    return np.ascontiguousarray(out, dtype=np.float32)
